# revision 13
# baseline (speedup 1.0000x reference)
"""Trainium2 Bass kernel for DenseLayerWithComplexNeurons.

Reference computation (B=8, S=1024, DIN=1024, DOUT=1024, A=4, T=4, H=8):
    z = x @ W.T + bias                      # (B,S, A*DOUT)
    z -> (B,S,T,G,A), G = DOUT//T = 256
    h = tanh(z @ cw1[t] + cb1[t])           # (B,S,T,G,H)
    o = h @ cw2[t] + cb2[t]                 # (B,S,T,G) -> (B,S,DOUT)

Sharding: 8 cores = 4 token blocks (2048 tokens each) x 2 feature halves
(2048 W-rows / 512 neurons each).  All compute runs in a transposed layout
(features on partitions, tokens on the free dim) so the tiny per-neuron
MLPs become small constant matmuls on the tensor engine:
  - expansion E[t]: (g,a) -> (g,h) block-diagonal with cw1
  - reduction S[t]: (g,h) -> (g)   block-diagonal with cw2

v2 notes (vs the f32r baseline at ~222us):
  - all matmul operands are bf16: same 1 cycle/row PE rate as f32r but
    half the HBM traffic, half the SBUF footprint and 2x faster
    LDWEIGHTS via fast-weight-load, so weight loads stay hidden under
    the 213ns matmul streams.
  - the linear bias is added by the DVE during the PSUM->SBUF copy of z
    (per-partition scalar), so the tanh bias reduces to cb1 which is
    constant per cell type; one ACTIVATE then covers a whole expansion
    pair via a 2-bank PSUM tile.
  - inputs stream as per-partition-contiguous blocks spread over 4 DMA
    rings; warmup starts at ~4us off a gpsimd memset instead of ~8us.
  - output is written bf16 (halves output traffic; host casts back).
"""

import numpy as np
import ml_dtypes

import concourse.bass as bass  # noqa: F401  (bass types via bacc)
import concourse.mybir as mybir
import concourse.tile as tile
from concourse import bacc
from concourse.bass_utils import run_bass_kernel_spmd

F32 = mybir.dt.float32
BF16 = mybir.dt.bfloat16
BF16_NP = ml_dtypes.bfloat16

B, S, DIN, DOUT, A, T, H = 8, 1024, 1024, 1024, 4, 4, 8
G = DOUT // T                     # 256 neurons per cell type
NTOK = B * S                      # 8192 tokens
DP, TP = 4, 2                     # token blocks x feature halves
TOK_C = NTOK // DP                # 2048 tokens per core
NRN_C = DOUT // TP                # 512 neurons per core
FEAT_C = A * NRN_C                # 2048 A-expanded features per core
KC = DIN // 128                   # 8 contraction chunks
NB = TOK_C // 512                 # 4 token sub-blocks per core
FC = FEAT_C // 128                # 16 feature chunks per core

_NC_CACHE = []


def _build_nc():
    nc = bacc.Bacc("TRN2", target_bir_lowering=False, debug=False, num_devices=8)

    xT = nc.declare_dram_parameter("xT", [128, NB, KC, 512], BF16, isOutput=False)
    wT = nc.declare_dram_parameter("wT", [128, FC, KC, 128], BF16, isOutput=False)
    eM = nc.declare_dram_parameter("eM", [128, 2, 128], BF16, isOutput=False)
    sM = nc.declare_dram_parameter("sM", [128, 2, 8, 128], BF16, isOutput=False)
    bl = nc.declare_dram_parameter("bl", [128, FC], F32, isOutput=False)
    c1 = nc.declare_dram_parameter("c1", [128, 2], F32, isOutput=False)
    c2 = nc.declare_dram_parameter("c2", [128, 2], F32, isOutput=False)
    oT = nc.declare_dram_parameter("oT", [NRN_C, TOK_C], BF16, isOutput=True)

    with tile.TileContext(nc) as tc:
        with tc.tile_pool(name="wp", bufs=1) as wp, \
             tc.tile_pool(name="cst", bufs=1) as cst, \
             tc.tile_pool(name="xp", bufs=4) as xp, \
             tc.tile_pool(name="zb", bufs=4) as zb, \
             tc.tile_pool(name="tb", bufs=6) as tb, \
             tc.tile_pool(name="ob", bufs=3) as ob, \
             tc.tile_pool(name="zp", bufs=2, space="PSUM") as zp, \
             tc.tile_pool(name="hp", bufs=2, space="PSUM") as hp, \
             tc.tile_pool(name="op", bufs=2, space="PSUM") as op:

            # --- PE warm-up off a gpsimd memset (gpsimd is idle at start,
            # so the warm-up matmuls can begin right after the entry
            # barrier while every DMA ring streams inputs).
            wu = cst.tile([128, 512], BF16, tag="wu")
            nc.gpsimd.memset(wu[:], 0.001)
            wu_f = cst.tile([128, 16], F32, tag="wuf")
            nc.gpsimd.memset(wu_f[:], 0.001)
            # warmups live in the hp pool so the opening z chain's PSUM
            # tile (zp pool) carries no dependency on the warmup finishing
            for _ in range(10):
                wu_ps = hp.tile([128, 1024], F32, tag="h")
                nc.tensor.matmul(wu_ps[:, 0:512], wu[:, 0:128], wu[:],
                                 start=True, stop=True)

            # --- input tiles + DMA plan.  Everything is per-partition
            # contiguous in DRAM; spread across 4 rings so x0 and w0 (the
            # first z chain's gates) land concurrently.
            x_tiles = [xp.tile([128, KC, 512], BF16, tag="x", name=f"x_{nb}")
                       for nb in range(NB)]
            w_all = wp.tile([128, FC, KC, 128], BF16, tag="w")
            e_sb = cst.tile([128, 2, 128], BF16, tag="e")
            s_sb = cst.tile([128, 2, 8, 128], BF16, tag="s")
            bl_sb = cst.tile([128, FC], F32, tag="bl")
            c1_sb = cst.tile([128, 2], F32, tag="c1")
            c2_sb = cst.tile([128, 2], F32, tag="c2")

            # inputs go on the two HWDGE rings only (the gpsimd ring is a
            # slow software-descriptor path — v2 measured ~30us latency on
            # it, which gated the whole pipeline)
            # x0 is split across BOTH HWDGE rings in parallel (k0-3 on
            # sync, k4-7 leading the scalar ring) — halves its arrival
            # time and hedges the per-ring queue-startup jitter that
            # bounced first-data between ~11us and ~17us across runs
            nc.sync.dma_start(x_tiles[0][:, 0:4], xT[:, 0, 0:4])
            nc.sync.dma_start(w_all[:, 0], wT[:, 0])
            for fc in range(1, 8):
                nc.sync.dma_start(w_all[:, fc], wT[:, fc])
            nc.sync.dma_start(x_tiles[2][:], xT[:, 2])
            nc.scalar.dma_start(x_tiles[0][:, 4:KC], xT[:, 0, 4:KC])
            nc.scalar.dma_start(bl_sb[:], bl[:])         # then consts,
            nc.scalar.dma_start(e_sb[:], eM[:])          # w8..15, x1
            nc.scalar.dma_start(s_sb[:], sM[:])
            nc.scalar.dma_start(c1_sb[:], c1[:])
            nc.scalar.dma_start(c2_sb[:], c2[:])
            for fc in range(8, FC):
                nc.scalar.dma_start(w_all[:, fc], wT[:, fc])
            nc.scalar.dma_start(x_tiles[1][:], xT[:, 1])
            nc.gpsimd.dma_start(x_tiles[3][:], xT[:, 3])  # gpsimd: x3 only
            # tanh table preload once the scalar ring's issues are queued
            nc.scalar.activation(wu_f[:, 0:8], wu_f[:, 8:16],
                                 mybir.ActivationFunctionType.Tanh)

            # Software-pipelined epilogue: expansion runs ~2 chunks behind
            # the main z matmuls and reduction ~3 behind, so the PE never
            # waits on the DVE bias-add or the ScalarE tanh.
            o_tiles = {}                       # (nb, grp) -> o_ps psum tile
            e_stage = []                       # awaiting expansion
            s_stage = []                       # awaiting reduction

            def emit_expansion(item):
                nb, grp, q, z_sb = item
                tl = grp // 2
                # K=64 row-tiled pair: base_partition 0/64 auto-derives
                # tile_position; the pair pipelines back-to-back.  The two
                # halves land in one 2-bank PSUM tile so a single tanh
                # ACTIVATE (bias = cb1, constant per cell type) covers both.
                h_ps = hp.tile([128, 1024], F32, tag="h")
                for half in range(2):
                    nc.tensor.matmul(
                        h_ps[:, bass.ds(half * 512, 512)],
                        e_sb[bass.ds(half * 64, 64), tl, :],
                        z_sb[bass.ds(half * 64, 64), :],
                        start=True, stop=True)
                th = tb.tile([128, 1024], BF16, tag="t")
                nc.scalar.activation(th[:], h_ps[:],
                                     mybir.ActivationFunctionType.Tanh,
                                     bias=c1_sb[:, bass.ds(tl, 1)])
                s_stage.append((nb, grp, q, th))

            def emit_reduction(item):
                nb, grp, q, th = item
                tl = grp // 2
                if q == 0:
                    o_tiles[(nb, grp)] = op.tile(
                        [128, 512], F32, tag="o", name=f"ops_{nb}_{grp}")
                o_ps = o_tiles[(nb, grp)]
                for half in range(2):
                    rr = q * 2 + half
                    nc.tensor.matmul(
                        o_ps[:], s_sb[:, tl, rr, :],
                        th[:, bass.ds(half * 512, 512)],
                        start=(rr == 0), stop=(rr == 7))
                if q == 3:
                    o_sb = ob.tile([128, 512], BF16, tag="o")
                    last = (nb == NB - 1 and grp == 3)
                    dst = oT[bass.ds(grp * 128, 128), bass.ds(nb * 512, 512)]
                    if last:
                        # tail: split the add across ScalarE/DVE and the DMA
                        # across the low-latency sync + scalar rings
                        nc.scalar.activation(
                            o_sb[:, 0:256], o_ps[:, 0:256],
                            mybir.ActivationFunctionType.Identity,
                            bias=c2_sb[:, bass.ds(tl, 1)])
                        nc.vector.tensor_scalar_add(
                            o_sb[:, 256:512], o_ps[:, 256:512],
                            c2_sb[:, bass.ds(tl, 1)])
                        nc.sync.dma_start(
                            oT[bass.ds(grp * 128, 128),
                               bass.ds(nb * 512, 256)], o_sb[:, 0:256])
                        nc.scalar.dma_start(
                            oT[bass.ds(grp * 128, 128),
                               bass.ds(nb * 512 + 256, 256)], o_sb[:, 256:512])
                    else:
                        nc.vector.tensor_scalar_add(
                            o_sb[:], o_ps[:], c2_sb[:, bass.ds(tl, 1)])
                        # sync ring is idle after the input burst; HWDGE
                        # completes ~10x sooner than the gpsimd SW ring
                        nc.sync.dma_start(dst, o_sb[:])
                    del o_tiles[(nb, grp)]

            for nb in range(NB):
                x_nb = x_tiles[nb]
                for grp in range(4):
                    for q in range(4):
                        fc = grp * 4 + q
                        z_ps = zp.tile([128, 512], F32, tag="z")
                        for k in range(KC):
                            nc.tensor.matmul(
                                z_ps[:],
                                w_all[:, fc, k, :],
                                x_nb[:, k, :],
                                start=(k == 0), stop=(k == KC - 1))
                        # PSUM -> SBUF with the linear bias folded in
                        # (per-partition scalar), output already bf16
                        z_sb = zb.tile([128, 512], BF16, tag="z")
                        nc.vector.tensor_scalar_add(
                            z_sb[:], z_ps[:], bl_sb[:, bass.ds(fc, 1)])

                        if len(s_stage) >= 2:
                            emit_reduction(s_stage.pop(0))
                        if len(e_stage) >= 2:
                            emit_expansion(e_stage.pop(0))
                        e_stage.append((nb, grp, q, z_sb))
                        if nb == NB - 1 and grp == 3:
                            # shrink the pipeline lag through the final
                            # group so little work trails the last z chain
                            if len(s_stage) >= 2:
                                emit_reduction(s_stage.pop(0))
                            if len(e_stage) >= 2:
                                emit_expansion(e_stage.pop(0))

            while e_stage or s_stage:
                if s_stage:
                    emit_reduction(s_stage.pop(0))
                if e_stage:
                    emit_expansion(e_stage.pop(0))

    nc.compile()
    return nc


def _host_prep(x, weight, bias, cw1, cb1, cw2, cb2):
    """Build the 8 per-core input maps (all host-side numpy)."""
    x2 = np.ascontiguousarray(x, dtype=np.float32).reshape(NTOK, DIN)
    weight = np.asarray(weight, dtype=np.float32)
    bias = np.asarray(bias, dtype=np.float32)
    cw1 = np.asarray(cw1, dtype=np.float32)   # (T, A, H)
    cb1 = np.asarray(cb1, dtype=np.float32)   # (T, H)
    cw2 = np.asarray(cw2, dtype=np.float32)   # (T, H)
    cb2 = np.asarray(cb2, dtype=np.float32)   # (T,)

    # xT[p, nb, k, n] = x2[tok0 + nb*512 + n, k*128 + p]
    xT_all = []
    for i in range(DP):
        blk = x2[i * TOK_C:(i + 1) * TOK_C]            # (TOK_C, DIN)
        t = blk.reshape(NB, 512, KC, 128).transpose(3, 0, 2, 1)
        xT_all.append(np.ascontiguousarray(t).astype(BF16_NP))

    # wT[p, fc, k, f] = W[j*FEAT_C + fc*128 + f, k*128 + p]
    wT_all = []
    for j in range(TP):
        wj = weight[j * FEAT_C:(j + 1) * FEAT_C]       # (FEAT_C, DIN)
        t = wj.reshape(FC, 128, KC, 128).transpose(3, 0, 2, 1)
        wT_all.append(np.ascontiguousarray(t).astype(BF16_NP))

    # E[t]: (g*4+a, g16*8+h) block-diag cw1; S[t]: (g*8+h, g') block-diag cw2
    e_all, s_all, bl_all, c1_all, c2_all = [], [], [], [], []
    for j in range(TP):
        eMm = np.zeros((128, 2, 128), np.float32)
        sMm = np.zeros((128, 2, 8, 128), np.float32)
        for tl in range(2):
            t = 2 * j + tl
            for g16 in range(16):   # K=64 expansion block, doubled on rows
                for a in range(A):
                    for h in range(H):
                        v = cw1[t, a, h]
                        eMm[g16 * 4 + a, tl, g16 * 8 + h] = v
                        eMm[64 + g16 * 4 + a, tl, g16 * 8 + h] = v
            for rr in range(8):
                for g in range(16):
                    for h in range(H):
                        sMm[g * 8 + h, tl, rr, rr * 16 + g] = cw2[t, h]
        e_all.append(eMm.astype(BF16_NP))
        s_all.append(sMm.astype(BF16_NP))

        # linear bias per z chunk: bl[p, fc] = bias[j*FEAT_C + fc*128 + p]
        bl_all.append(np.ascontiguousarray(
            bias[j * FEAT_C:(j + 1) * FEAT_C].reshape(FC, 128).T))

        # tanh bias: cb1 per h_ps partition (p = g16*8 + h -> h = p % 8)
        c1m = np.zeros((128, 2), np.float32)
        c2m = np.zeros((128, 2), np.float32)
        for tl in range(2):
            c1m[:, tl] = np.tile(cb1[2 * j + tl], 16)
            c2m[:, tl] = cb2[2 * j + tl]
        c1_all.append(c1m)
        c2_all.append(c2m)

    in_maps = []
    for c in range(8):
        i, j = c // TP, c % TP
        in_maps.append({
            "xT": xT_all[i], "wT": wT_all[j], "eM": e_all[j],
            "sM": s_all[j], "bl": bl_all[j], "c1": c1_all[j],
            "c2": c2_all[j],
        })
    return in_maps


def kernel(x, weight, bias, cw1, cb1, cw2, cb2):
    in_maps = _host_prep(x, weight, bias, cw1, cb1, cw2, cb2)
    if not _NC_CACHE:
        _NC_CACHE.append(_build_nc())
    nc = _NC_CACHE[0]
    try:
        res = run_bass_kernel_spmd(nc, in_maps, list(range(8)))
    except Exception:
        # transient NRT device faults have been observed once after crashed
        # runs; a clean retry in the same process recovers
        res = run_bass_kernel_spmd(nc, in_maps, list(range(8)))
    out = np.empty((NTOK, DOUT), np.float32)
    for c in range(8):
        i, j = c // TP, c % TP
        oc = np.asarray(res.results[c]["oT"]).astype(np.float32)
        out[i * TOK_C:(i + 1) * TOK_C, j * NRN_C:(j + 1) * NRN_C] = oc.T
    return out.reshape(B, S, DOUT)


# revision 16
# speedup vs baseline: 1.1748x; 1.1748x over previous
"""Trainium2 Bass kernel for DenseLayerWithComplexNeurons.

Reference computation (B=8, S=1024, DIN=1024, DOUT=1024, A=4, T=4, H=8):
    z = x @ W.T + bias                      # (B,S, A*DOUT)
    z -> (B,S,T,G,A), G = DOUT//T = 256
    h = tanh(z @ cw1[t] + cb1[t])           # (B,S,T,G,H)
    o = h @ cw2[t] + cb2[t]                 # (B,S,T,G) -> (B,S,DOUT)

Sharding: 8 cores = 4 token blocks (2048 tokens each) x 2 feature halves
(2048 W-rows / 512 neurons each).  All compute runs in a transposed layout
(features on partitions, tokens on the free dim) so the tiny per-neuron
MLPs become small constant matmuls on the tensor engine:
  - expansion E[t]: (g,a) -> (g,h) block-diagonal with cw1
  - reduction S[t]: (g,h) -> (g)   block-diagonal with cw2

v2 notes (vs the f32r baseline at ~222us):
  - all matmul operands are bf16: same 1 cycle/row PE rate as f32r but
    half the HBM traffic, half the SBUF footprint and 2x faster
    LDWEIGHTS via fast-weight-load, so weight loads stay hidden under
    the 213ns matmul streams.
  - the linear bias is added by the DVE during the PSUM->SBUF copy of z
    (per-partition scalar), so the tanh bias reduces to cb1 which is
    constant per cell type; one ACTIVATE then covers a whole expansion
    pair via a 2-bank PSUM tile.
  - inputs stream as per-partition-contiguous blocks spread over 4 DMA
    rings; warmup starts at ~4us off a gpsimd memset instead of ~8us.
  - output is written bf16 (halves output traffic; host casts back).
"""

import numpy as np
import ml_dtypes

import concourse.bass as bass  # noqa: F401  (bass types via bacc)
import concourse.mybir as mybir
import concourse.tile as tile
from concourse import bacc
from concourse.bass_utils import run_bass_kernel_spmd

F32 = mybir.dt.float32
BF16 = mybir.dt.bfloat16
BF16_NP = ml_dtypes.bfloat16

B, S, DIN, DOUT, A, T, H = 8, 1024, 1024, 1024, 4, 4, 8
G = DOUT // T                     # 256 neurons per cell type
NTOK = B * S                      # 8192 tokens
DP, TP = 4, 2                     # token blocks x feature halves
TOK_C = NTOK // DP                # 2048 tokens per core
NRN_C = DOUT // TP                # 512 neurons per core
FEAT_C = A * NRN_C                # 2048 A-expanded features per core
KC = DIN // 128                   # 8 contraction chunks
NB = TOK_C // 512                 # 4 token sub-blocks per core
FC = FEAT_C // 128                # 16 feature chunks per core

_NC_CACHE = []


def _build_nc():
    nc = bacc.Bacc("TRN2", target_bir_lowering=False, debug=False, num_devices=8)

    xT = nc.declare_dram_parameter("xT", [128, NB, KC, 512], BF16, isOutput=False)
    wT = nc.declare_dram_parameter("wT", [128, FC, KC, 128], BF16, isOutput=False)
    eM = nc.declare_dram_parameter("eM", [128, 2, 128], BF16, isOutput=False)
    sM = nc.declare_dram_parameter("sM", [128, 2, 8, 128], BF16, isOutput=False)
    bl = nc.declare_dram_parameter("bl", [128, FC], F32, isOutput=False)
    c1 = nc.declare_dram_parameter("c1", [128, 2], F32, isOutput=False)
    c2 = nc.declare_dram_parameter("c2", [128, 2], F32, isOutput=False)
    oT = nc.declare_dram_parameter("oT", [NRN_C, TOK_C], BF16, isOutput=True)

    with tile.TileContext(nc) as tc:
        with tc.tile_pool(name="wp", bufs=1) as wp, \
             tc.tile_pool(name="cst", bufs=1) as cst, \
             tc.tile_pool(name="xp", bufs=4) as xp, \
             tc.tile_pool(name="zb", bufs=4) as zb, \
             tc.tile_pool(name="tb", bufs=6) as tb, \
             tc.tile_pool(name="ob", bufs=3) as ob, \
             tc.tile_pool(name="zp", bufs=2, space="PSUM") as zp, \
             tc.tile_pool(name="hp", bufs=2, space="PSUM") as hp, \
             tc.tile_pool(name="op", bufs=2, space="PSUM") as op:

            # --- PE warm-up off a gpsimd memset (gpsimd is idle at start,
            # so the warm-up matmuls can begin right after the entry
            # barrier while every DMA ring streams inputs).
            wu = cst.tile([128, 512], BF16, tag="wu")
            nc.gpsimd.memset(wu[:], 0.001)
            wu_f = cst.tile([128, 16], F32, tag="wuf")
            nc.gpsimd.memset(wu_f[:], 0.001)
            # warmups live in the hp pool so the opening z chain's PSUM
            # tile (zp pool) carries no dependency on the warmup finishing
            # N=256 warmups: the in-order tensor queue drains them before
            # the first real chain, so total warmup length directly delays
            # the start — 12x256 ends ~11.6us (>=3us busy, full clock)
            # right as the x0a+w0 DMAs typically land
            for _ in range(12):
                wu_ps = zp.tile([128, 512], F32, tag="z")
                nc.tensor.matmul(wu_ps[:, 0:256], wu[:, 0:128], wu[:, 0:256],
                                 start=True, stop=True)

            # --- input tiles + DMA plan.  Everything is per-partition
            # contiguous in DRAM; spread across 4 rings so x0 and w0 (the
            # first z chain's gates) land concurrently.
            x_tiles = [xp.tile([128, KC, 512], BF16, tag="x", name=f"x_{nb}")
                       for nb in range(NB)]
            w_all = wp.tile([128, FC, KC, 128], BF16, tag="w")
            e_sb = cst.tile([128, 2, 128], BF16, tag="e")
            s_sb = cst.tile([128, 2, 8, 128], BF16, tag="s")
            bl_sb = cst.tile([128, FC], F32, tag="bl")
            c1_sb = cst.tile([128, 2], F32, tag="c1")
            c2_sb = cst.tile([128, 2], F32, tag="c2")

            # inputs go on the two HWDGE rings only (the gpsimd ring is a
            # slow software-descriptor path — v2 measured ~30us latency on
            # it, which gated the whole pipeline)
            # x0 k0-3 then w0 land first so the opening z chain can begin
            # while x0's k4-7 half is still streaming (all on the sync
            # ring: splitting x0 across rings regressed — the k4-7 chain
            # stalls mid-group when the second ring lags)
            nc.sync.dma_start(x_tiles[0][:, 0:4], xT[:, 0, 0:4])
            nc.sync.dma_start(w_all[:, 0], wT[:, 0])
            nc.sync.dma_start(x_tiles[0][:, 4:KC], xT[:, 0, 4:KC])
            for fc in range(1, 8):
                nc.sync.dma_start(w_all[:, fc], wT[:, fc])
            nc.sync.dma_start(x_tiles[2][:], xT[:, 2])
            nc.scalar.dma_start(bl_sb[:], bl[:])         # scalar: consts,
            nc.scalar.dma_start(e_sb[:], eM[:])          # w8..15, x1
            nc.scalar.dma_start(s_sb[:], sM[:])
            nc.scalar.dma_start(c1_sb[:], c1[:])
            nc.scalar.dma_start(c2_sb[:], c2[:])
            for fc in range(8, FC):
                nc.scalar.dma_start(w_all[:, fc], wT[:, fc])
            nc.scalar.dma_start(x_tiles[1][:], xT[:, 1])
            nc.gpsimd.dma_start(x_tiles[3][:], xT[:, 3])  # gpsimd: x3 only
            # tanh table preload once the scalar ring's issues are queued
            nc.scalar.activation(wu_f[:, 0:8], wu_f[:, 8:16],
                                 mybir.ActivationFunctionType.Tanh)

            # Software-pipelined epilogue: expansion runs ~2 chunks behind
            # the main z matmuls and reduction ~3 behind, so the PE never
            # waits on the DVE bias-add or the ScalarE tanh.
            o_tiles = {}                       # (nb, grp) -> o_ps psum tile
            e_stage = []                       # awaiting expansion
            s_stage = []                       # awaiting reduction

            def emit_expansion(item):
                nb, grp, q, z_sb = item
                tl = grp // 2
                # K=64 row-tiled pair: base_partition 0/64 auto-derives
                # tile_position; the pair pipelines back-to-back.  The two
                # halves land in one 2-bank PSUM tile so a single tanh
                # ACTIVATE (bias = cb1, constant per cell type) covers both.
                h_ps = hp.tile([128, 1024], F32, tag="h")
                for half in range(2):
                    nc.tensor.matmul(
                        h_ps[:, bass.ds(half * 512, 512)],
                        e_sb[bass.ds(half * 64, 64), tl, :],
                        z_sb[bass.ds(half * 64, 64), :],
                        start=True, stop=True)
                th = tb.tile([128, 1024], BF16, tag="t")
                nc.scalar.activation(th[:], h_ps[:],
                                     mybir.ActivationFunctionType.Tanh,
                                     bias=c1_sb[:, bass.ds(tl, 1)])
                s_stage.append((nb, grp, q, th))

            def emit_reduction(item):
                nb, grp, q, th = item
                tl = grp // 2
                if q == 0:
                    o_tiles[(nb, grp)] = op.tile(
                        [128, 512], F32, tag="o", name=f"ops_{nb}_{grp}")
                o_ps = o_tiles[(nb, grp)]
                for half in range(2):
                    rr = q * 2 + half
                    nc.tensor.matmul(
                        o_ps[:], s_sb[:, tl, rr, :],
                        th[:, bass.ds(half * 512, 512)],
                        start=(rr == 0), stop=(rr == 7))
                if q == 3:
                    o_sb = ob.tile([128, 512], BF16, tag="o")
                    last = (nb == NB - 1 and grp == 3)
                    dst = oT[bass.ds(grp * 128, 128), bass.ds(nb * 512, 512)]
                    if last:
                        # tail: split the add across ScalarE/DVE and the DMA
                        # across the low-latency sync + scalar rings
                        nc.scalar.activation(
                            o_sb[:, 0:256], o_ps[:, 0:256],
                            mybir.ActivationFunctionType.Identity,
                            bias=c2_sb[:, bass.ds(tl, 1)])
                        nc.vector.tensor_scalar_add(
                            o_sb[:, 256:512], o_ps[:, 256:512],
                            c2_sb[:, bass.ds(tl, 1)])
                        nc.sync.dma_start(
                            oT[bass.ds(grp * 128, 128),
                               bass.ds(nb * 512, 256)], o_sb[:, 0:256])
                        nc.scalar.dma_start(
                            oT[bass.ds(grp * 128, 128),
                               bass.ds(nb * 512 + 256, 256)], o_sb[:, 256:512])
                    else:
                        nc.vector.tensor_scalar_add(
                            o_sb[:], o_ps[:], c2_sb[:, bass.ds(tl, 1)])
                        # sync ring is idle after the input burst; HWDGE
                        # completes ~10x sooner than the gpsimd SW ring
                        nc.sync.dma_start(dst, o_sb[:])
                    del o_tiles[(nb, grp)]

            for nb in range(NB):
                x_nb = x_tiles[nb]
                for grp in range(4):
                    for q in range(4):
                        fc = grp * 4 + q
                        z_ps = zp.tile([128, 512], F32, tag="z")
                        for k in range(KC):
                            nc.tensor.matmul(
                                z_ps[:],
                                w_all[:, fc, k, :],
                                x_nb[:, k, :],
                                start=(k == 0), stop=(k == KC - 1))
                        # PSUM -> SBUF with the linear bias folded in
                        # (per-partition scalar), output already bf16
                        z_sb = zb.tile([128, 512], BF16, tag="z")
                        nc.vector.tensor_scalar_add(
                            z_sb[:], z_ps[:], bl_sb[:, bass.ds(fc, 1)])

                        if len(s_stage) >= 2:
                            emit_reduction(s_stage.pop(0))
                        if len(e_stage) >= 2:
                            emit_expansion(e_stage.pop(0))
                        e_stage.append((nb, grp, q, z_sb))
                        if nb == NB - 1 and grp == 3:
                            # shrink the pipeline lag through the final
                            # group so little work trails the last z chain
                            if len(s_stage) >= 2:
                                emit_reduction(s_stage.pop(0))
                            if len(e_stage) >= 2:
                                emit_expansion(e_stage.pop(0))

            while e_stage or s_stage:
                if s_stage:
                    emit_reduction(s_stage.pop(0))
                if e_stage:
                    emit_expansion(e_stage.pop(0))

    nc.compile()
    return nc


def _host_prep(x, weight, bias, cw1, cb1, cw2, cb2):
    """Build the 8 per-core input maps (all host-side numpy)."""
    x2 = np.ascontiguousarray(x, dtype=np.float32).reshape(NTOK, DIN)
    weight = np.asarray(weight, dtype=np.float32)
    bias = np.asarray(bias, dtype=np.float32)
    cw1 = np.asarray(cw1, dtype=np.float32)   # (T, A, H)
    cb1 = np.asarray(cb1, dtype=np.float32)   # (T, H)
    cw2 = np.asarray(cw2, dtype=np.float32)   # (T, H)
    cb2 = np.asarray(cb2, dtype=np.float32)   # (T,)

    # xT[p, nb, k, n] = x2[tok0 + nb*512 + n, k*128 + p]
    xT_all = []
    for i in range(DP):
        blk = x2[i * TOK_C:(i + 1) * TOK_C]            # (TOK_C, DIN)
        t = blk.reshape(NB, 512, KC, 128).transpose(3, 0, 2, 1)
        xT_all.append(np.ascontiguousarray(t).astype(BF16_NP))

    # wT[p, fc, k, f] = W[j*FEAT_C + fc*128 + f, k*128 + p]
    wT_all = []
    for j in range(TP):
        wj = weight[j * FEAT_C:(j + 1) * FEAT_C]       # (FEAT_C, DIN)
        t = wj.reshape(FC, 128, KC, 128).transpose(3, 0, 2, 1)
        wT_all.append(np.ascontiguousarray(t).astype(BF16_NP))

    # E[t]: (g*4+a, g16*8+h) block-diag cw1; S[t]: (g*8+h, g') block-diag cw2
    e_all, s_all, bl_all, c1_all, c2_all = [], [], [], [], []
    for j in range(TP):
        eMm = np.zeros((128, 2, 128), np.float32)
        sMm = np.zeros((128, 2, 8, 128), np.float32)
        for tl in range(2):
            t = 2 * j + tl
            for g16 in range(16):   # K=64 expansion block, doubled on rows
                for a in range(A):
                    for h in range(H):
                        v = cw1[t, a, h]
                        eMm[g16 * 4 + a, tl, g16 * 8 + h] = v
                        eMm[64 + g16 * 4 + a, tl, g16 * 8 + h] = v
            for rr in range(8):
                for g in range(16):
                    for h in range(H):
                        sMm[g * 8 + h, tl, rr, rr * 16 + g] = cw2[t, h]
        e_all.append(eMm.astype(BF16_NP))
        s_all.append(sMm.astype(BF16_NP))

        # linear bias per z chunk: bl[p, fc] = bias[j*FEAT_C + fc*128 + p]
        bl_all.append(np.ascontiguousarray(
            bias[j * FEAT_C:(j + 1) * FEAT_C].reshape(FC, 128).T))

        # tanh bias: cb1 per h_ps partition (p = g16*8 + h -> h = p % 8)
        c1m = np.zeros((128, 2), np.float32)
        c2m = np.zeros((128, 2), np.float32)
        for tl in range(2):
            c1m[:, tl] = np.tile(cb1[2 * j + tl], 16)
            c2m[:, tl] = cb2[2 * j + tl]
        c1_all.append(c1m)
        c2_all.append(c2m)

    in_maps = []
    for c in range(8):
        i, j = c // TP, c % TP
        in_maps.append({
            "xT": xT_all[i], "wT": wT_all[j], "eM": e_all[j],
            "sM": s_all[j], "bl": bl_all[j], "c1": c1_all[j],
            "c2": c2_all[j],
        })
    return in_maps


def kernel(x, weight, bias, cw1, cb1, cw2, cb2):
    in_maps = _host_prep(x, weight, bias, cw1, cb1, cw2, cb2)
    if not _NC_CACHE:
        _NC_CACHE.append(_build_nc())
    nc = _NC_CACHE[0]
    try:
        res = run_bass_kernel_spmd(nc, in_maps, list(range(8)))
    except Exception:
        # transient NRT device faults have been observed once after crashed
        # runs; a clean retry in the same process recovers
        res = run_bass_kernel_spmd(nc, in_maps, list(range(8)))
    out = np.empty((NTOK, DOUT), np.float32)
    for c in range(8):
        i, j = c // TP, c % TP
        oc = np.asarray(res.results[c]["oT"]).astype(np.float32)
        out[i * TOK_C:(i + 1) * TOK_C, j * NRN_C:(j + 1) * NRN_C] = oc.T
    return out.reshape(B, S, DOUT)
